# revision 12
# baseline (speedup 1.0000x reference)
"""Trainium2 Bass kernel for nn_Evaluate_6158983102660 (retrieval_knn).

Pipeline per core (8 cores = 2 batches x 4 hw-quarters, 2304 pixels each):
  1. DVE: offsets -> per-candidate flat rf-row indices (int16, slab-local)
  2. gpsimd dma_gather: per candidate (pixel,k) fetch its 512B rf row from DRAM
  3. DVE: dot(lf[pixel], gathered row) via tensor_tensor mult + segmented reduce
  4. ACT exp, PE ones-matmul partition-sum -> per-k partial softmax denominators
  5. AllReduce (4-core groups = one batch) of the 48 denominators
  6. probs = e * (1/Z); DVE max8/max_index/match_replace top-16 per pixel
  7. iota-eq extraction of the packed (iy,ix) for the top-16; decompose to
     output offsets on device.
Host only reshapes/shards inputs and reassembles full outputs.
"""
import sys

if '/opt/trn_rl_repo' not in sys.path:
    sys.path.insert(0, '/opt/trn_rl_repo')

import numpy as np

import concourse.bass as bass
import concourse.bacc as bacc
import concourse.mybir as mybir
from concourse.bass_utils import run_bass_kernel_spmd
from concourse.tile import TileContext
from contextlib import ExitStack

B, C, H, W = 2, 128, 96, 96
NQUEUES = 4
GSPLIT = 6              # sub-gathers per tile (ring capacity caps ~1024 idxs)
HW = H * W
NUM = 48
K = 16
NQ = 4                  # hw quarters per batch
QROWS = H // NQ         # 24 rows per quarter
TILES = 18              # 3 x 6 blocks of 8x16 pixels
TP = 128                # pixels per tile
SLABROWS = 39           # rf slab rows per core: [24q-8, 24q+31)
SLABCOLS = SLABROWS * W # 3744
WSLOTS = 3              # ceil(18/8) tile slots per 16-partition group in W layout
WFREE = WSLOTS * 384    # 1152
F32 = mybir.dt.float32
I16 = mybir.dt.int16
I32 = mybir.dt.int32
U32 = mybir.dt.uint32

_nc_cache = {}


def _pixel_grids():
    """Y[p,t], X[p,t]: global image coords for partition p of tile t (quarter-rel rows added later)."""
    p = np.arange(TP)
    t = np.arange(TILES)
    br, bc = t // 6, t % 6
    y = 8 * br[None, :] + (p // 16)[:, None]     # quarter-relative row
    x = 16 * bc[None, :] + (p % 16)[:, None]
    return y, x                                   # (128, 18) each


def _build_nc(n_cores):
    key = n_cores
    if key in _nc_cache:
        return _nc_cache[key]

    nc = bacc.Bacc(num_swdge_queues=NQUEUES)
    rfrows_d = nc.declare_dram_parameter("rfrows", [SLABCOLS, C], F32, isOutput=False)
    lft_d = nc.declare_dram_parameter("lft", [TILES * TP, C], F32, isOutput=False)
    oxp_d = nc.declare_dram_parameter("oxp", [TP, TILES * NUM], F32, isOutput=False)
    oyp_d = nc.declare_dram_parameter("oyp", [TP, TILES * NUM], F32, isOutput=False)
    oxw_d = nc.declare_dram_parameter("oxw", [TP, WFREE], F32, isOutput=False)
    oyw_d = nc.declare_dram_parameter("oyw", [TP, WFREE], F32, isOutput=False)
    ytp_d = nc.declare_dram_parameter("ytabp", [TP, TILES], F32, isOutput=False)
    xtp_d = nc.declare_dram_parameter("xtabp", [TP, TILES], F32, isOutput=False)
    ytw_d = nc.declare_dram_parameter("ytabw", [TP, WFREE], F32, isOutput=False)
    xtw_d = nc.declare_dram_parameter("xtabw", [TP, WFREE], F32, isOutput=False)
    rlo_d = nc.declare_dram_parameter("rowlo", [TP, 1], F32, isOutput=False)
    io48_d = nc.declare_dram_parameter("iota48", [TP, NUM], F32, isOutput=False)

    corr_o = nc.declare_dram_parameter("corro", [TP, TILES * K], F32, isOutput=True)
    oxo_o = nc.declare_dram_parameter("oxo", [TP, TILES * K], F32, isOutput=True)
    oyo_o = nc.declare_dram_parameter("oyo", [TP, TILES * K], F32, isOutput=True)

    zin_b = nc.dram_tensor("zin_b", [1, NUM], F32)
    zout_b = nc.dram_tensor("zout_b", [1, NUM], F32)

    groups = [[0, 1, 2, 3], [4, 5, 6, 7]] if n_cores == 8 else [[0]]

    AL = mybir.AluOpType
    AF = mybir.ActivationFunctionType

    with TileContext(nc) as tc, ExitStack() as ctx:
        sb = ctx.enter_context(tc.tile_pool(name="sb", bufs=1))
        gp = ctx.enter_context(tc.tile_pool(name="gp", bufs=2))
        ps = ctx.enter_context(tc.tile_pool(name="ps", bufs=1, space="PSUM"))

        # ---------- load everything ----------
        lft = sb.tile([TP, TILES * C], F32)     # [p, t, c] per-pixel lf rows
        nc.sync.dma_start(
            out=lft[:].rearrange("p (t c) -> p t c", t=TILES),
            in_=lft_d[:].rearrange("(t p) c -> p t c", p=TP))
        oxp = sb.tile([TP, TILES * NUM], F32)
        nc.sync.dma_start(out=oxp[:], in_=oxp_d[:])
        oyp = sb.tile([TP, TILES * NUM], F32)
        nc.sync.dma_start(out=oyp[:], in_=oyp_d[:])
        oxw = sb.tile([TP, WFREE], F32)
        nc.sync.dma_start(out=oxw[:], in_=oxw_d[:])
        oyw = sb.tile([TP, WFREE], F32)
        nc.sync.dma_start(out=oyw[:], in_=oyw_d[:])
        ytp = sb.tile([TP, TILES], F32)
        nc.sync.dma_start(out=ytp[:], in_=ytp_d[:])
        xtp = sb.tile([TP, TILES], F32)
        nc.sync.dma_start(out=xtp[:], in_=xtp_d[:])
        ytw = sb.tile([TP, WFREE], F32)
        nc.sync.dma_start(out=ytw[:], in_=ytw_d[:])
        xtw = sb.tile([TP, WFREE], F32)
        nc.sync.dma_start(out=xtw[:], in_=xtw_d[:])
        rlo = sb.tile([TP, 1], F32)
        nc.sync.dma_start(out=rlo[:], in_=rlo_d[:])
        iota48 = sb.tile([TP, NUM], F32)
        nc.sync.dma_start(out=iota48[:], in_=io48_d[:])

        # ---------- W-layout index math -> gather indices ----------
        iyw = sb.tile([TP, WFREE], F32)
        nc.vector.tensor_tensor(out=iyw[:], in0=oyw[:], in1=ytw[:], op=AL.add)
        nc.vector.tensor_scalar(out=iyw[:], in0=iyw[:], scalar1=0.0, scalar2=95.0,
                                op0=AL.max, op1=AL.min)
        ixw = sb.tile([TP, WFREE], F32)
        nc.vector.tensor_tensor(out=ixw[:], in0=oxw[:], in1=xtw[:], op=AL.add)
        nc.vector.tensor_scalar(out=ixw[:], in0=ixw[:], scalar1=0.0, scalar2=95.0,
                                op0=AL.max, op1=AL.min)
        # local row = iy - rowlo; ind = local_row*96 + ix
        nc.vector.tensor_tensor(out=iyw[:], in0=iyw[:],
                                in1=rlo[:].broadcast_to([TP, WFREE]), op=AL.subtract)
        nc.vector.tensor_scalar(out=iyw[:], in0=iyw[:], scalar1=float(W), scalar2=None,
                                op0=AL.mult)
        nc.vector.tensor_tensor(out=iyw[:], in0=iyw[:], in1=ixw[:], op=AL.add)
        indw = sb.tile([TP, WFREE], I16)
        nc.vector.tensor_copy(out=indw[:], in_=iyw[:])

        # replicate each tile's (16,384) index block to all 16-partition bands
        wall = sb.tile([TP, TILES * 384], I16)
        for t in range(TILES):
            g, s = t % 8, t // 8
            src = indw[16 * g:16 * (g + 1), s * 384:(s + 1) * 384]
            for band in range(8):
                nc.sync.dma_start(
                    out=wall[16 * band:16 * (band + 1), t * 384:(t + 1) * 384],
                    in_=src)

        # ---------- p-layout pack2 (iy*256+ix, global coords) ----------
        iyp = sb.tile([TP, TILES * NUM], F32)
        nc.vector.tensor_tensor(
            out=iyp[:].rearrange("p (t k) -> p t k", t=TILES),
            in0=oyp[:].rearrange("p (t k) -> p t k", t=TILES),
            in1=ytp[:].unsqueeze(2).broadcast_to([TP, TILES, NUM]), op=AL.add)
        nc.vector.tensor_scalar(out=iyp[:], in0=iyp[:], scalar1=0.0, scalar2=95.0,
                                op0=AL.max, op1=AL.min)
        ixp = sb.tile([TP, TILES * NUM], F32)
        nc.vector.tensor_tensor(
            out=ixp[:].rearrange("p (t k) -> p t k", t=TILES),
            in0=oxp[:].rearrange("p (t k) -> p t k", t=TILES),
            in1=xtp[:].unsqueeze(2).broadcast_to([TP, TILES, NUM]), op=AL.add)
        nc.vector.tensor_scalar(out=ixp[:], in0=ixp[:], scalar1=0.0, scalar2=95.0,
                                op0=AL.max, op1=AL.min)
        pack2 = sb.tile([TP, TILES * NUM], F32)
        nc.vector.tensor_scalar(out=pack2[:], in0=iyp[:], scalar1=256.0, scalar2=None,
                                op0=AL.mult)
        nc.vector.tensor_tensor(out=pack2[:], in0=pack2[:], in1=ixp[:], op=AL.add)

        # ---------- main loop: gather + dot + exp + Z accumulation ----------
        e_all = sb.tile([TP, TILES * NUM], F32)
        m_all = sb.tile([TP, TILES * NUM], F32)
        ones_col = sb.tile([TP, 1], F32)
        nc.vector.memset(ones_col[:], 1.0)
        zps = ps.tile([1, NUM], F32, space="PSUM")

        for t in range(TILES):
            g_t = gp.tile([TP, NUM * C], F32, tag="gath")
            nsub = TP * NUM // GSPLIT          # 1024 idxs per sub-gather
            kchunk = NUM // GSPLIT             # 8 k-chunks per sub-gather
            for j in range(GSPLIT):
                nc.gpsimd.dma_gather(
                    out_ap=g_t[:].rearrange("p (k c) -> p k c", k=NUM)
                        [:, j * kchunk:(j + 1) * kchunk, :],
                    in_ap=rfrows_d[:],
                    idxs_ap=wall[:, t * 384 + j * (nsub // 16):
                                 t * 384 + (j + 1) * (nsub // 16)],
                    num_idxs=nsub,
                    num_idxs_reg=nsub,
                    elem_size=C,
                    queue_num=(t * GSPLIT + j) % NQUEUES,
                )
            nc.vector.tensor_tensor(
                out=g_t[:].rearrange("p (k c) -> p k c", k=NUM),
                in0=g_t[:].rearrange("p (k c) -> p k c", k=NUM),
                in1=lft[:].rearrange("p (t c) -> p t c", t=TILES)[:, t:t + 1, :]
                    .broadcast_to([TP, NUM, C]),
                op=AL.mult)
            nc.vector.tensor_reduce(
                out=m_all[:, t * NUM:(t + 1) * NUM],
                in_=g_t[:].rearrange("p (k c) -> p k c", k=NUM),
                axis=mybir.AxisListType.X, op=AL.add)
            nc.scalar.activation(
                out=e_all[:, t * NUM:(t + 1) * NUM],
                in_=m_all[:, t * NUM:(t + 1) * NUM], func=AF.Exp)
            nc.tensor.matmul(
                out=zps[:], lhsT=ones_col[:], rhs=e_all[:, t * NUM:(t + 1) * NUM],
                start=(t == 0), stop=(t == TILES - 1))

        # ---------- AllReduce Z over the batch group ----------
        z_sb = sb.tile([1, NUM], F32)
        nc.vector.tensor_copy(out=z_sb[:], in_=zps[:])
        nc.sync.dma_start(out=zin_b[:], in_=z_sb[:])
        nc.gpsimd.collective_compute(
            "AllReduce", AL.add, replica_groups=groups,
            ins=[zin_b[:]], outs=[zout_b[:]])
        zar = sb.tile([1, NUM], F32)
        nc.sync.dma_start(out=zar[:], in_=zout_b[:])
        rz = sb.tile([1, NUM], F32)
        nc.vector.reciprocal(rz[:], zar[:])
        ones_row = sb.tile([1, TP], F32)
        nc.vector.memset(ones_row[:], 1.0)
        rzps = ps.tile([TP, NUM], F32, space="PSUM")
        nc.tensor.matmul(out=rzps[:], lhsT=ones_row[:], rhs=rz[:], start=True, stop=True)
        rzb = sb.tile([TP, NUM], F32)
        nc.vector.tensor_copy(out=rzb[:], in_=rzps[:])

        probs = sb.tile([TP, TILES * NUM], F32)
        nc.vector.tensor_tensor(
            out=probs[:].rearrange("p (t k) -> p t k", t=TILES),
            in0=e_all[:].rearrange("p (t k) -> p t k", t=TILES),
            in1=rzb[:].unsqueeze(1).broadcast_to([TP, TILES, NUM]), op=AL.mult)

        # ---------- top-16 of 48 per pixel ----------
        corr_sb = sb.tile([TP, TILES * K], F32)
        topi_f = sb.tile([TP, TILES * K], F32)
        for t in range(TILES):
            wrk = gp.tile([TP, NUM], F32, tag="wrk")
            idx_u = gp.tile([TP, 8], U32, tag="idxu")
            pt = probs[:, t * NUM:(t + 1) * NUM]
            c8a = corr_sb[:, t * K:t * K + 8]
            c8b = corr_sb[:, t * K + 8:t * K + 16]
            nc.vector.max(out=c8a, in_=pt)
            nc.vector.max_index(out=idx_u[:], in_max=c8a, in_values=pt)
            nc.vector.tensor_copy(out=topi_f[:, t * K:t * K + 8], in_=idx_u[:])
            nc.vector.match_replace(out=wrk[:], in_to_replace=c8a, in_values=pt,
                                    imm_value=-1.0)
            nc.vector.max(out=c8b, in_=wrk[:])
            nc.vector.max_index(out=idx_u[:], in_max=c8b, in_values=wrk[:])
            nc.vector.tensor_copy(out=topi_f[:, t * K + 8:t * K + 16], in_=idx_u[:])

        # ---------- gather pack2 at topi via iota-eq + reduce ----------
        eqt = sb.tile([TP, TILES * K * NUM], F32)
        nc.vector.tensor_tensor(
            out=eqt[:].rearrange("p (t r k) -> p t r k", t=TILES, r=K),
            in0=topi_f[:].rearrange("p (t r) -> p t r", t=TILES).unsqueeze(3)
                .broadcast_to([TP, TILES, K, NUM]),
            in1=iota48[:].unsqueeze(1).unsqueeze(1).broadcast_to([TP, TILES, K, NUM]),
            op=AL.is_equal)
        nc.vector.tensor_tensor(
            out=eqt[:].rearrange("p (t r k) -> p t r k", t=TILES, r=K),
            in0=eqt[:].rearrange("p (t r k) -> p t r k", t=TILES, r=K),
            in1=pack2[:].rearrange("p (t k) -> p t k", t=TILES).unsqueeze(2)
                .broadcast_to([TP, TILES, K, NUM]),
            op=AL.mult)
        sel = sb.tile([TP, TILES * K], F32)
        nc.vector.tensor_reduce(
            out=sel[:], in_=eqt[:].rearrange("p (t r k) -> p t r k", t=TILES, r=K),
            axis=mybir.AxisListType.X, op=AL.add)

        # ---------- decompose sel -> output offsets ----------
        seli = sb.tile([TP, TILES * K], F32)
        nc.vector.tensor_scalar(out=seli[:], in0=sel[:], scalar1=1.0 / 256.0,
                                scalar2=None, op0=AL.mult)
        seli32 = sb.tile([TP, TILES * K], I32)
        nc.vector.tensor_copy(out=seli32[:], in_=seli[:])
        iyf = sb.tile([TP, TILES * K], F32)
        nc.vector.tensor_copy(out=iyf[:], in_=seli32[:])
        ixf = sb.tile([TP, TILES * K], F32)
        nc.vector.tensor_scalar(out=ixf[:], in0=iyf[:], scalar1=-256.0, scalar2=None,
                                op0=AL.mult)
        nc.vector.tensor_tensor(out=ixf[:], in0=ixf[:], in1=sel[:], op=AL.add)
        oxout = sb.tile([TP, TILES * K], F32)
        nc.vector.tensor_tensor(
            out=oxout[:].rearrange("p (t r) -> p t r", t=TILES),
            in0=ixf[:].rearrange("p (t r) -> p t r", t=TILES),
            in1=xtp[:].unsqueeze(2).broadcast_to([TP, TILES, K]), op=AL.subtract)
        oyout = sb.tile([TP, TILES * K], F32)
        nc.vector.tensor_tensor(
            out=oyout[:].rearrange("p (t r) -> p t r", t=TILES),
            in0=iyf[:].rearrange("p (t r) -> p t r", t=TILES),
            in1=ytp[:].unsqueeze(2).broadcast_to([TP, TILES, K]), op=AL.subtract)

        nc.sync.dma_start(out=corr_o[:], in_=corr_sb[:])
        nc.sync.dma_start(out=oxo_o[:], in_=oxout[:])
        nc.sync.dma_start(out=oyo_o[:], in_=oyout[:])

    if not nc.is_finalized():
        nc.finalize()
    _nc_cache[key] = nc
    return nc


def _host_prep(inputs, n_cores):
    lf = np.ascontiguousarray(inputs["left_features"], dtype=np.float32)
    rf = np.ascontiguousarray(inputs["right_features"], dtype=np.float32)
    ox = np.ascontiguousarray(inputs["offset_x"], dtype=np.float32)
    oy = np.ascontiguousarray(inputs["offset_y"], dtype=np.float32)

    yq, xg = _pixel_grids()      # quarter-relative Y (128,18), global X (128,18)
    in_maps = []
    for core in range(n_cores):
        b, q = core // NQ, core % NQ
        rowlo = QROWS * q - 8
        Y = (yq + QROWS * q).astype(np.int64)     # global rows (128,18)
        X = xg.astype(np.int64)

        # lft rows in (tile, pixel) order: row t*128+p = lf[b,:,pixel(Y[p,t],X[p,t])]
        lfbT = lf[b].T                                     # (9216, 128)
        lft = np.ascontiguousarray(
            lfbT[(Y * W + X).T.reshape(-1)])               # (18*128, 128)
        rfb = np.ascontiguousarray(rf[:, b * HW:(b + 1) * HW].T)   # (9216, 128)
        slab = np.zeros((SLABCOLS, C), np.float32)
        r0, r1 = max(0, rowlo), min(H, rowlo + SLABROWS)
        slab[(r0 - rowlo) * W:(r1 - rowlo) * W] = rfb[r0 * W:r1 * W]

        # p-layout offsets (128, 18, 48) -> flat (128, 18*48)
        oxb = ox[b][:, Y, X]                       # (48, 128, 18)
        oyb = oy[b][:, Y, X]
        oxp = np.ascontiguousarray(oxb.transpose(1, 2, 0).reshape(TP, -1))
        oyp = np.ascontiguousarray(oyb.transpose(1, 2, 0).reshape(TP, -1))

        # W-layout (wrapped for dma_gather): [16g+b2, s*384 + 8k + a]
        oxw = np.zeros((TP, WFREE), np.float32)
        oyw = np.zeros((TP, WFREE), np.float32)
        ytw = np.zeros((TP, WFREE), np.float32)
        xtw = np.zeros((TP, WFREE), np.float32)
        for t in range(TILES):
            g, s = t % 8, t // 8
            blkx = oxb[:, :, t].reshape(NUM, 8, 16).transpose(2, 0, 1).reshape(16, 384)
            blky = oyb[:, :, t].reshape(NUM, 8, 16).transpose(2, 0, 1).reshape(16, 384)
            oxw[16 * g:16 * (g + 1), s * 384:(s + 1) * 384] = blkx
            oyw[16 * g:16 * (g + 1), s * 384:(s + 1) * 384] = blky
            yblk = np.broadcast_to(Y[:, t].reshape(8, 16).T[:, None, :], (16, NUM, 8))
            xblk = np.broadcast_to(X[:, t].reshape(8, 16).T[:, None, :], (16, NUM, 8))
            ytw[16 * g:16 * (g + 1), s * 384:(s + 1) * 384] = yblk.reshape(16, 384)
            xtw[16 * g:16 * (g + 1), s * 384:(s + 1) * 384] = xblk.reshape(16, 384)

        in_maps.append({
            "rfrows": slab,
            "lft": lft,
            "oxp": oxp, "oyp": oyp,
            "oxw": oxw, "oyw": oyw,
            "ytabp": Y.astype(np.float32), "xtabp": X.astype(np.float32),
            "ytabw": ytw, "xtabw": xtw,
            "rowlo": np.full((TP, 1), float(rowlo), np.float32),
            "iota48": np.broadcast_to(np.arange(NUM, dtype=np.float32), (TP, NUM)).copy(),
        })
    return in_maps


def _assemble(results, n_cores):
    ox_out = np.zeros((B, K, H, W), np.float32)
    oy_out = np.zeros((B, K, H, W), np.float32)
    corr = np.zeros((B, K, HW), np.float32)
    yq, xg = _pixel_grids()
    for core in range(n_cores):
        b, q = core // NQ, core % NQ
        Y = yq + QROWS * q
        X = xg
        flat = Y * W + X
        carr = results[core]["corro"].reshape(TP, TILES, K)
        xarr = results[core]["oxo"].reshape(TP, TILES, K)
        yarr = results[core]["oyo"].reshape(TP, TILES, K)
        for r in range(K):
            corr[b, r, flat] = carr[:, :, r]
            ox_out[b, r, Y, X] = xarr[:, :, r]
            oy_out[b, r, Y, X] = yarr[:, :, r]
    return ox_out, oy_out, corr


def kernel(**inputs):
    n_cores = 8
    nc = _build_nc(n_cores)
    in_maps = _host_prep(inputs, n_cores)
    res = run_bass_kernel_spmd(nc, in_maps, list(range(n_cores)))
    return _assemble(res.results, n_cores)
